# revision 21
# baseline (speedup 1.0000x reference)
"""Trainium2 Bass kernel for nn_AssigmentLayer (8-core data-parallel).

Math (B=131072, T=30, F=10, MAX_LEN=30, K=10 shifts):
  x_c = inputs[:, 0, c] for c in {0,1};  rc_c[m] = x_c[m//30] * w_c[m%30]
  out[b, j, 2i+c] = rc_c[j*B + b - i]   (0 for negative index), i in [0,10)
  out[b, j, 20+t] = inputs[b, j, 2+t],  t in [0,8)

Sharding: batch dim b split contiguously across 8 cores (B8=16384 each).

Per core, for each (j, c), the needed rc values form one contiguous
segment seg[r=2j+c][t] = rc_c[m_base_j + t], m_base_j = j*B + s*B8 - 9.
Stage 1 computes the 60 segment rows into a persistent SBUF tile as
  seg = xA*wsa + xB*wsb
where xA/xB are step-0-broadcast views of a host-gathered compact x
table (the +1 batch shift and the masked/rotated w tables absorb the
per-row mod-30 phase; the host does pure index gathers, no arithmetic).

Stage 2 processes groups of 512 output rows mapped b = g*512 + 4p + v
(p = SBUF partition, v = sub-tile slot).  All (v, i) shift pairs with
equal d = v - i need the same data, so 13 PE transpose-matmuls (lhsT =
stride-4 slices of the segment rows, identity rhs) serve all 40
combinations; PSUM-bank-aligned slots let two strided copies per slot
scatter them into the interleaved output tile, the tail features are
staged and interleaved by the other copy engine, and each partition
stores 4 complete consecutive output rows as one contiguous 13.4 KB
descriptor (full 128-partition, ~HBM-line-rate stores).

Measured: ~198-230 us/NEFF on 8 cores, bitwise-exact vs the reference
(70.8 MB of HBM traffic/core ~= the 358 GB/s per-core roofline).
"""

import sys

import numpy as np

if "/opt/trn_rl_repo" not in sys.path:
    sys.path.insert(0, "/opt/trn_rl_repo")

B = 131072
T = 30
NCORES = 8
B8 = B // NCORES            # 16384
TILE_P = 128                # output rows per sub-tile (exact tiling)
GRP = 4                     # sub-tiles per group (128 = 32*4)
NCHUNK = 6
CHA = 92                    # batches per segment chunk
CHW = CHA * 30              # 2760 floats per chunk
SEGW = NCHUNK * CHW         # 16560 (>= 16393 needed)
XCW = NCHUNK * CHA + 4      # 556

_CACHE = {}


def _build_nc():
    import concourse.bacc as bacc
    import concourse.tile as tile
    from concourse import mybir
    from contextlib import ExitStack

    f32 = mybir.dt.float32
    nc = bacc.Bacc("TRN2", target_bir_lowering=False, debug=False,
                   num_devices=NCORES)

    tail_in = nc.declare_dram_parameter("tail", [B8, T, 8], f32, isOutput=False)
    xc_in = nc.declare_dram_parameter("xcomp", [60, XCW], f32, isOutput=False)
    wa_in = nc.declare_dram_parameter("wsa", [60, 30], f32, isOutput=False)
    wb_in = nc.declare_dram_parameter("wsb", [60, 30], f32, isOutput=False)
    id_in = nc.declare_dram_parameter("ident", [60, 60], f32, isOutput=False)
    out_ext = nc.declare_dram_parameter("out", [B8, T, 28], f32, isOutput=True)

    with tile.TileContext(nc) as tc:
        with ExitStack() as ctx:
            const_pool = ctx.enter_context(tc.tile_pool(name="const", bufs=1))
            seg_pool = ctx.enter_context(tc.tile_pool(name="seg", bufs=1))
            xw_pool = ctx.enter_context(tc.tile_pool(name="xw", bufs=2))
            ps2_pool = ctx.enter_context(
                tc.tile_pool(name="ps2", bufs=4, space="PSUM"))
            out_pool = ctx.enter_context(tc.tile_pool(name="outp", bufs=4))
            tailp = ctx.enter_context(tc.tile_pool(name="tailp", bufs=3))

            # all loads ride the Sync-engine HWDGE ring (SP has no startup
            # table-load, so its first trigger lands ~3us before ACT's);
            # Scalar then only runs copies, GpSimd only SWDGE store emission.
            ident = const_pool.tile([60, 60], f32)
            nc.sync.dma_start(ident[:], id_in[:])
            xcomp = const_pool.tile([60, XCW], f32)
            nc.sync.dma_start(xcomp[:], xc_in[:])
            wsa = const_pool.tile([60, 30], f32)
            nc.sync.dma_start(wsa[:], wa_in[:])
            wsb = const_pool.tile([60, 30], f32)
            nc.sync.dma_start(wsb[:], wb_in[:])
            wA = wsa[:].unsqueeze(1).broadcast_to((60, CHA, 30))
            wB = wsb[:].unsqueeze(1).broadcast_to((60, CHA, 30))

            # persistent segment rows: seg[2j+c, t] = rc_c[m_base_j + t]
            segsb = seg_pool.tile([60, SEGW], f32)

            def emit_chunk(a0, na):
                # seg[r, 30a+e] = x[mb0+a]*wsa[e] + x[mb0+a+1]*wsb[e]
                xA = xcomp[:, a0:a0 + na]
                xA = xA.unsqueeze(-1).broadcast_to((60, na, 30))
                xB = xcomp[:, a0 + 1:a0 + na + 1]
                xB = xB.unsqueeze(-1).broadcast_to((60, na, 30))
                wAn = wsa[:].unsqueeze(1).broadcast_to((60, na, 30))
                wBn = wsb[:].unsqueeze(1).broadcast_to((60, na, 30))
                sv = segsb[:, a0 * 30:(a0 + na) * 30].rearrange(
                    "p (a e) -> p a e", e=30)
                tmp = xw_pool.tile([60, CHW], f32, tag="tmp")
                tv = tmp[:, 0:na * 30].rearrange("p (a e) -> p a e", e=30)
                nc.vector.tensor_mul(sv, xA, wAn)
                nc.vector.tensor_mul(tv, xB, wBn)
                nc.vector.tensor_add(
                    segsb[:, a0 * 30:(a0 + na) * 30],
                    segsb[:, a0 * 30:(a0 + na) * 30], tmp[:, 0:na * 30])

            GR = GRP * TILE_P            # rows per group (512)
            LB = 4                       # groups per batched tail load

            # Tail loads ride the Sync/SP HWDGE ring alone, batched LB groups
            # per dma_start (one ~0.6us sequencer trigger enqueues 512
            # descriptors), all emitted upfront: the DMA engines saturate on
            # load backlog from ~8us while the first group's compute chain is
            # still running, and the load ring independently fills any gaps
            # the store ring leaves (one shared ring couples them: a copy
            # hiccup then drains the only backlog -> v5 decayed after 60us).
            # The tailp pool (bufs=3) gates prefetch depth; nothing else runs
            # on SP so its stalls are harmless.
            tstg4s = {}

            def emit_load(k):
                t4 = tailp.tile([128, LB * 240 * GRP], f32, tag="tstg4")
                src = tail_in[k * LB * GR:(k + 1) * LB * GR]
                src = src.rearrange("(w p v) j t -> p w v (j t)", w=LB,
                                    v=GRP)
                dst = t4[:].rearrange("p (w v f) -> p w v f", w=LB, v=GRP)
                nc.sync.dma_start(dst, src)
                tstg4s[k] = t4

            for k in range(B8 // GR // LB):
                emit_load(k)

            def emit_group(g):
                # rows of this group: b = g*GR + 4*p + v  (p partition, v slot)
                otile = out_pool.tile([128, 840 * GRP], f32, tag="otile")
                tstg = tstg4s[g // LB][:, (g % LB) * 240 * GRP:
                                       (g % LB + 1) * 240 * GRP]
                # 13 distinct shifted transposes serve all (v, i) pairs:
                # value for (v,i) depends only on d = v - i in [-9, 3].
                # slot s = 3 - d at psum cols [64s, 64s+60) (bank-aligned).
                ps = ps2_pool.tile([128, 832], f32, tag="ps2")
                for d in range(-9, GRP):
                    s = (GRP - 1) - d
                    base = g * GR + 9 + d
                    nc.tensor.transpose(
                        ps[:, 64 * s:64 * s + 60],
                        segsb[:, base:base + GRP * 127 + 1:GRP],
                        ident[:],
                    )
                psv = ps[:].rearrange("p (s x) -> p s x", x=64)
                for v in range(GRP):
                    ovw = otile[:, 840 * v:840 * (v + 1)]
                    od = ovw.rearrange("p (j i c) -> p j i c", j=30, i=14, c=2)
                    # src col for (j, i, c) = 64*(3 - v + i) + 2j + c;
                    # split the copy at the PSUM bank boundary (slot 8)
                    s0 = (GRP - 1) - v
                    isplit = 8 - s0          # i < isplit -> slots 3-v..7
                    src = psv[:, s0:s0 + 10, :]
                    src = src.rearrange("p i (j c) -> p j i c", c=2)[
                        :, 0:30, :, :]
                    # Scalar gets only v=0's pieces: its stream also pushes
                    # the store triggers, so it needs ~3us of slack per
                    # group for the ACT store ring to never run dry.
                    teng = nc.scalar.copy if v == 0 else nc.vector.tensor_copy
                    teng(od[:, :, 0:isplit, :], src[:, :, 0:isplit, :])
                    teng(od[:, :, isplit:10, :], src[:, :, isplit:10, :])
                    # tail interleave
                    ts = tstg[:, 240 * v:240 * (v + 1)]
                    nc.vector.tensor_copy(
                        ovw.rearrange("p (j k) -> p j k", j=30)[:, :, 20:28],
                        ts.rearrange("p (j t) -> p j t", j=30),
                    )
                dst = out_ext[g * GR:(g + 1) * GR]
                dst = dst.rearrange("(p v) j k -> p v (j k)", v=GRP)
                src = otile[:].rearrange("p (v f) -> p v f", v=GRP)
                # f32 HWDGE store on the ACT ring (measured 464 ns/desc vs
                # 497 on the SP ring).  SWDGE is avoided: its descriptor-ring
                # port contention makes DMA engine 15 ~20% slower, a 19-26us
                # straggler at the end.
                nc.scalar.dma_start(dst, src)

            # emission order interleaves segment chunks with groups so the
            # scheduler can overlap stage 1 with early stage-2 groups.
            # chunks are emitted just-in-time: one 23-batch chunk unblocks
            # group 0; later chunks slot between groups (small + uniform to
            # avoid Vector bursts that starve the store DMA).
            chunks = [23] * 24                    # batches; sum = 552
            bounds = []
            acc = 0
            for na in chunks:
                bounds.append((acc, na))
                acc += na
            assert acc * 30 == SEGW
            ngroups = B8 // GR           # 32
            done_c = 0
            covered = 0                  # floats of segsb ready
            for g in range(ngroups):
                need = g * GR + GRP - 1 + 9 + GRP * 127 + 1
                while covered < min(need, SEGW) and done_c < len(bounds):
                    a0, na = bounds[done_c]
                    emit_chunk(a0, na)
                    covered = (a0 + na) * 30
                    done_c += 1
                emit_group(g)
            while done_c < len(bounds):
                a0, na = bounds[done_c]
                emit_chunk(a0, na)
                done_c += 1

    nc.compile()
    return nc


def _get_nc():
    if "nc" not in _CACHE:
        _CACHE["nc"] = _build_nc()
    return _CACHE["nc"]


def _prep_core(inputs, w1, w2, s):
    """Per-core input map: pure index gathers, no arithmetic."""
    f32 = np.float32
    x01 = inputs[:, 0, 0:2]                     # (B, 2)
    PAD = 2
    xpad = np.zeros((PAD + B + XCW + 4, 2), dtype=f32)
    xpad[PAD:PAD + B] = x01
    xcomp = np.zeros((60, XCW), dtype=f32)
    wsa = np.zeros((60, 30), dtype=f32)
    wsb = np.zeros((60, 30), dtype=f32)
    w = [np.asarray(w1, f32).reshape(T), np.asarray(w2, f32).reshape(T)]
    e = np.arange(30)
    for c in range(2):
        for j in range(T):
            m_base = j * B + s * B8 - 9
            mb0 = m_base // 30
            o = m_base - 30 * mb0
            r = 2 * j + c
            xcomp[r] = xpad[PAD + mb0:PAD + mb0 + XCW, c]
            wv = w[c][(o + e) % 30]
            wsa[r] = np.where(o + e < 30, wv, 0.0)
            wsb[r] = np.where(o + e >= 30, wv, 0.0)
    tail = np.ascontiguousarray(inputs[s * B8:(s + 1) * B8, :, 2:])
    return {
        "tail": tail,
        "xcomp": xcomp,
        "wsa": wsa,
        "wsb": wsb,
        "ident": np.eye(60, dtype=f32),
    }


def _run(inputs, w1, w2, trace=False, trace_kwargs=None):
    from concourse.bass_utils import run_bass_kernel_spmd

    nc = _get_nc()
    inputs = np.asarray(inputs, dtype=np.float32)
    in_maps = [_prep_core(inputs, w1, w2, s) for s in range(NCORES)]
    res = run_bass_kernel_spmd(
        nc, in_maps, core_ids=list(range(NCORES)), trace=trace,
        **(trace_kwargs or {}),
    )
    out = np.concatenate(
        [res.results[i]["out"] for i in range(NCORES)], axis=0)
    return out, res


def kernel(inputs, w1, w2):
    return _run(inputs, w1, w2)[0]



# revision 26
# speedup vs baseline: 1.0687x; 1.0687x over previous
"""Trainium2 Bass kernel for nn_AssigmentLayer (8-core data-parallel).

Math (B=131072, T=30, F=10, MAX_LEN=30, K=10 shifts):
  x_c = inputs[:, 0, c] for c in {0,1};  rc_c[m] = x_c[m//30] * w_c[m%30]
  out[b, j, 2i+c] = rc_c[j*B + b - i]   (0 for negative index), i in [0,10)
  out[b, j, 20+t] = inputs[b, j, 2+t],  t in [0,8)

Sharding: batch dim b split contiguously across 8 cores (B8=16384 each).

Per core, for each (j, c), the needed rc values form one contiguous
segment seg[r=2j+c][t] = rc_c[m_base_j + t], m_base_j = j*B + s*B8 - 9.
Stage 1 computes the 60 segment rows into a persistent SBUF tile as
  seg = xA*wsa + xB*wsb
where xA/xB are step-0-broadcast views of a host-gathered compact x
table (the +1 batch shift and the masked/rotated w tables absorb the
per-row mod-30 phase; the host does pure index gathers, no arithmetic).

Stage 2 processes groups of 512 output rows mapped b = g*512 + 4p + v
(p = SBUF partition, v = sub-tile slot).  All (v, i) shift pairs with
equal d = v - i need the same data, so 13 PE transpose-matmuls (lhsT =
stride-4 slices of the segment rows, identity rhs) serve all 40
combinations; PSUM-bank-aligned slots let two strided copies per slot
scatter them into the interleaved output tile, the tail features are
staged and interleaved by the other copy engine, and each partition
stores 4 complete consecutive output rows as one contiguous 13.4 KB
descriptor (full 128-partition, ~HBM-line-rate stores).

Measured: ~198-230 us/NEFF on 8 cores, bitwise-exact vs the reference
(70.8 MB of HBM traffic/core ~= the 358 GB/s per-core roofline).
"""

import sys

import numpy as np

if "/opt/trn_rl_repo" not in sys.path:
    sys.path.insert(0, "/opt/trn_rl_repo")

B = 131072
T = 30
NCORES = 8
B8 = B // NCORES            # 16384
TILE_P = 128                # output rows per sub-tile (exact tiling)
GRP = 4                     # sub-tiles per group (128 = 32*4)
NCHUNK = 6
CHA = 92                    # batches per segment chunk
CHW = CHA * 30              # 2760 floats per chunk
SEGW = NCHUNK * CHW         # 16560 (>= 16393 needed)
XCW = NCHUNK * CHA + 4      # 556

_CACHE = {}


def _build_nc():
    import concourse.bacc as bacc
    import concourse.tile as tile
    from concourse import mybir
    from contextlib import ExitStack

    f32 = mybir.dt.float32
    nc = bacc.Bacc("TRN2", target_bir_lowering=False, debug=False,
                   num_devices=NCORES)

    tail_in = nc.declare_dram_parameter("tail", [B8, T, 8], f32, isOutput=False)
    xc_in = nc.declare_dram_parameter("xcomp", [60, XCW], f32, isOutput=False)
    wa_in = nc.declare_dram_parameter("wsa", [60, 30], f32, isOutput=False)
    wb_in = nc.declare_dram_parameter("wsb", [60, 30], f32, isOutput=False)
    id_in = nc.declare_dram_parameter("ident", [60, 60], f32, isOutput=False)
    out_ext = nc.declare_dram_parameter("out", [B8, T, 28], f32, isOutput=True)

    with tile.TileContext(nc) as tc:
        with ExitStack() as ctx:
            const_pool = ctx.enter_context(tc.tile_pool(name="const", bufs=1))
            seg_pool = ctx.enter_context(tc.tile_pool(name="seg", bufs=1))
            xw_pool = ctx.enter_context(tc.tile_pool(name="xw", bufs=2))
            ps2_pool = ctx.enter_context(
                tc.tile_pool(name="ps2", bufs=4, space="PSUM"))
            out_pool = ctx.enter_context(tc.tile_pool(name="outp", bufs=4))
            tailp = ctx.enter_context(tc.tile_pool(name="tailp", bufs=6))

            # all loads ride the Sync-engine HWDGE ring (SP has no startup
            # table-load, so its first trigger lands ~3us before ACT's);
            # Scalar then only runs copies, GpSimd only SWDGE store emission.
            ident = const_pool.tile([60, 60], f32)
            nc.sync.dma_start(ident[:], id_in[:])
            xcomp = const_pool.tile([60, XCW], f32)
            nc.sync.dma_start(xcomp[:], xc_in[:])
            wsa = const_pool.tile([60, 30], f32)
            nc.sync.dma_start(wsa[:], wa_in[:])
            wsb = const_pool.tile([60, 30], f32)
            nc.sync.dma_start(wsb[:], wb_in[:])
            wA = wsa[:].unsqueeze(1).broadcast_to((60, CHA, 30))
            wB = wsb[:].unsqueeze(1).broadcast_to((60, CHA, 30))

            # persistent segment rows: seg[2j+c, t] = rc_c[m_base_j + t]
            segsb = seg_pool.tile([60, SEGW], f32)

            def emit_chunk(a0, na):
                # seg[r, 30a+e] = x[mb0+a]*wsa[e] + x[mb0+a+1]*wsb[e]
                xA = xcomp[:, a0:a0 + na]
                xA = xA.unsqueeze(-1).broadcast_to((60, na, 30))
                xB = xcomp[:, a0 + 1:a0 + na + 1]
                xB = xB.unsqueeze(-1).broadcast_to((60, na, 30))
                wAn = wsa[:].unsqueeze(1).broadcast_to((60, na, 30))
                wBn = wsb[:].unsqueeze(1).broadcast_to((60, na, 30))
                sv = segsb[:, a0 * 30:(a0 + na) * 30].rearrange(
                    "p (a e) -> p a e", e=30)
                tmp = xw_pool.tile([60, CHW], f32, tag="tmp")
                tv = tmp[:, 0:na * 30].rearrange("p (a e) -> p a e", e=30)
                nc.vector.tensor_mul(sv, xA, wAn)
                nc.vector.tensor_mul(tv, xB, wBn)
                nc.vector.tensor_add(
                    segsb[:, a0 * 30:(a0 + na) * 30],
                    segsb[:, a0 * 30:(a0 + na) * 30], tmp[:, 0:na * 30])

            GR = GRP * TILE_P            # rows per group (512)
            LB = 2                       # groups per batched tail load

            # Tail loads on the ACT HWDGE ring, batched LB groups per
            # dma_start, with the first 6 batches (12 groups, ~6 MB) emitted
            # before any compute: the DMA engines saturate on load backlog
            # from ~8us while the first group's compute chain is still
            # running (in v2 the per-group JIT loads left the engines half
            # idle until the first store at ~27us).  Later batches are
            # interleaved after the group whose copies already satisfy their
            # tailp buffer-free wait, so they stall neither the Scalar
            # stream nor themselves (upfront-emitting ALL of them would
            # deadlock: the wait is on tail copies that sit later in the
            # same Scalar stream).  Stores get the SP ring to themselves —
            # the two rings are drained independently by the engines, so
            # load backlog fills any gap the store cadence leaves.
            tstg4s = {}

            def emit_load(k):
                t4 = tailp.tile([128, LB * 240 * GRP], f32, tag="tstg4")
                src = tail_in[k * LB * GR:(k + 1) * LB * GR]
                src = src.rearrange("(w p v) j t -> p w v (j t)", w=LB,
                                    v=GRP)
                dst = t4[:].rearrange("p (w v f) -> p w v f", w=LB, v=GRP)
                nc.scalar.dma_start(dst, src)
                tstg4s[k] = t4

            NLB = B8 // GR // LB         # 16 load batches
            for k in range(6):
                emit_load(k)

            def emit_group(g):
                # rows of this group: b = g*GR + 4*p + v  (p partition, v slot)
                otile = out_pool.tile([128, 840 * GRP], f32, tag="otile")
                tstg = tstg4s[g // LB][:, (g % LB) * 240 * GRP:
                                       (g % LB + 1) * 240 * GRP]
                # 13 distinct shifted transposes serve all (v, i) pairs:
                # value for (v,i) depends only on d = v - i in [-9, 3].
                # slot s = 3 - d at psum cols [64s, 64s+60) (bank-aligned).
                ps = ps2_pool.tile([128, 832], f32, tag="ps2")
                for d in range(-9, GRP):
                    s = (GRP - 1) - d
                    base = g * GR + 9 + d
                    nc.tensor.transpose(
                        ps[:, 64 * s:64 * s + 60],
                        segsb[:, base:base + GRP * 127 + 1:GRP],
                        ident[:],
                    )
                psv = ps[:].rearrange("p (s x) -> p s x", x=64)
                for v in range(GRP):
                    ovw = otile[:, 840 * v:840 * (v + 1)]
                    od = ovw.rearrange("p (j i c) -> p j i c", j=30, i=14, c=2)
                    # src col for (j, i, c) = 64*(3 - v + i) + 2j + c;
                    # split the copy at the PSUM bank boundary (slot 8)
                    s0 = (GRP - 1) - v
                    isplit = 8 - s0          # i < isplit -> slots 3-v..7
                    src = psv[:, s0:s0 + 10, :]
                    src = src.rearrange("p i (j c) -> p j i c", c=2)[
                        :, 0:30, :, :]
                    teng = nc.scalar.copy if v < 2 else nc.vector.tensor_copy
                    teng(od[:, :, 0:isplit, :], src[:, :, 0:isplit, :])
                    teng(od[:, :, isplit:10, :], src[:, :, isplit:10, :])
                    # tail interleave (opposite engine of the big copy)
                    ts = tstg[:, 240 * v:240 * (v + 1)]
                    teng2 = nc.vector.tensor_copy if v < 2 else nc.scalar.copy
                    teng2(
                        ovw.rearrange("p (j k) -> p j k", j=30)[:, :, 20:28],
                        ts.rearrange("p (j t) -> p j t", j=30),
                    )
                dst = out_ext[g * GR:(g + 1) * GR]
                dst = dst.rearrange("(p v) j k -> p v (j k)", v=GRP)
                src = otile[:].rearrange("p (v f) -> p v f", v=GRP)
                # f32 HWDGE store on the SP ring.  SP dispatches only store
                # triggers (huge slack), so the ring keeps up to 4 groups of
                # backlog and never runs dry while Scalar/Vector copy.
                # SWDGE is avoided: its descriptor-ring port contention makes
                # DMA engine 15 ~20% slower, a 19-26us straggler at the end.
                nc.sync.dma_start(dst, src)

            # emission order interleaves segment chunks with groups so the
            # scheduler can overlap stage 1 with early stage-2 groups.
            # chunks are emitted just-in-time: one 23-batch chunk unblocks
            # group 0; later chunks slot between groups (small + uniform to
            # avoid Vector bursts that starve the store DMA).
            chunks = [23] * 24                    # batches; sum = 552
            bounds = []
            acc = 0
            for na in chunks:
                bounds.append((acc, na))
                acc += na
            assert acc * 30 == SEGW
            ngroups = B8 // GR           # 32
            done_c = 0
            covered = 0                  # floats of segsb ready
            for g in range(ngroups):
                need = g * GR + GRP - 1 + 9 + GRP * 127 + 1
                while covered < min(need, SEGW) and done_c < len(bounds):
                    a0, na = bounds[done_c]
                    emit_chunk(a0, na)
                    covered = (a0 + na) * 30
                    done_c += 1
                emit_group(g)
                # next batched tail load: emitted right after the group
                # whose (already-emitted) tail copies satisfy its tailp
                # buffer-free wait, so the Scalar stream never stalls on it.
                if g % LB == LB - 1 and g // LB + 6 < NLB:
                    emit_load(g // LB + 6)
            while done_c < len(bounds):
                a0, na = bounds[done_c]
                emit_chunk(a0, na)
                done_c += 1

    nc.compile()
    return nc


def _get_nc():
    if "nc" not in _CACHE:
        _CACHE["nc"] = _build_nc()
    return _CACHE["nc"]


def _prep_core(inputs, w1, w2, s):
    """Per-core input map: pure index gathers, no arithmetic."""
    f32 = np.float32
    x01 = inputs[:, 0, 0:2]                     # (B, 2)
    PAD = 2
    xpad = np.zeros((PAD + B + XCW + 4, 2), dtype=f32)
    xpad[PAD:PAD + B] = x01
    xcomp = np.zeros((60, XCW), dtype=f32)
    wsa = np.zeros((60, 30), dtype=f32)
    wsb = np.zeros((60, 30), dtype=f32)
    w = [np.asarray(w1, f32).reshape(T), np.asarray(w2, f32).reshape(T)]
    e = np.arange(30)
    for c in range(2):
        for j in range(T):
            m_base = j * B + s * B8 - 9
            mb0 = m_base // 30
            o = m_base - 30 * mb0
            r = 2 * j + c
            xcomp[r] = xpad[PAD + mb0:PAD + mb0 + XCW, c]
            wv = w[c][(o + e) % 30]
            wsa[r] = np.where(o + e < 30, wv, 0.0)
            wsb[r] = np.where(o + e >= 30, wv, 0.0)
    tail = np.ascontiguousarray(inputs[s * B8:(s + 1) * B8, :, 2:])
    return {
        "tail": tail,
        "xcomp": xcomp,
        "wsa": wsa,
        "wsb": wsb,
        "ident": np.eye(60, dtype=f32),
    }


def _run(inputs, w1, w2, trace=False, trace_kwargs=None):
    from concourse.bass_utils import run_bass_kernel_spmd

    nc = _get_nc()
    inputs = np.asarray(inputs, dtype=np.float32)
    in_maps = [_prep_core(inputs, w1, w2, s) for s in range(NCORES)]
    res = run_bass_kernel_spmd(
        nc, in_maps, core_ids=list(range(NCORES)), trace=trace,
        **(trace_kwargs or {}),
    )
    out = np.concatenate(
        [res.results[i]["out"] for i in range(NCORES)], axis=0)
    return out, res


def kernel(inputs, w1, w2):
    return _run(inputs, w1, w2)[0]

